# revision 17
# baseline (speedup 1.0000x reference)
"""Trainium2 Bass kernel for nn_AttLSTM (attention-LSTM, K=4 steps).

Math per step (reference):
    a = softmax(h @ g_S.T, axis=1)            # [B, S]
    r = a @ g_S                               # [B, D]
    gates = f_x @ W_ih.T + b_ih + [h, r] @ W_hh.T + b_hh
    i, f, g, o = split(gates, 4)
    c' = sig(f)*c + sig(i)*tanh(g); h' = sig(o)*tanh(c') + f_x

Design (per core, data-parallel over batch: B_loc = 512 rows/core):
  - logits/gates matmuls in fp16 (f32 PSUM accumulation); the attention
    readout r = p @ g in fp8e4 with MatmulPerfMode.DoubleRow (2 k-tiles
    per instruction, 2x PE rate). Softmax probabilities quantize to fp8
    benignly (peaked distribution: logit std ~ sqrt(D) = 22.6).
  - x @ W_ih.T + biases precomputed once (x == f_x every step) -> xw.
  - g_S kept two ways: transposed [D, S] resident in SBUF fp16 (g_T, rhs
    of the logits matmul) and natural [S, D] streamed per step from a
    DRAM fp8 scratch copy (rhs of the DoubleRow readout matmul).
  - ALL transposes via PE transpose-mode in groups of [128,128] blocks
    into one PSUM bank + one strided copy back to SBUF.
  - softmax per 128-row b-tile: per-512-chunk negated max (DVE, from
    PSUM), exp with per-chunk bias straight from PSUM (ACT, fp8 out) +
    accum_out row-sums, then a global per-row rescale p *= exp(m_chunk -
    m_row) before use.
  - logits/gates matmuls emitted in PSUM-bank *pairs* with the
    contraction loop outermost so consecutive instructions share lhsT
    (halves LDWEIGHTS traffic when the compiler reuses loaded weights).
  - prolog interleaved with step 0: the A-phase (logits+softmax) of
    b-tiles 0 and 1 is emitted chunk-by-chunk between the g_S group
    loads/transposes, so the PE never idles waiting for DMA.
  - sigmoid computed as 0.5*tanh(x/2)+0.5 so the single `exp_and_others`
    ACT table set (Exp + Tanh) serves the whole kernel.
  - LSTM pointwise math as fused scalar_tensor_tensor ops on DVE,
    carrying z = 2c as state.
"""

import os
import sys

import numpy as np

for _p in ("/opt/trn_rl_repo",):
    if _p not in sys.path and os.path.isdir(_p):
        sys.path.insert(0, _p)

# Problem sizes (hardcoded per spec).
B, S, D = 4096, 8192, 512
H = D
N_CORES = 8
B_LOC = B // N_CORES          # 512 rows per core
K_STEPS = 4
P = 128                       # partitions


def build_bass(b_loc=B_LOC, s=S, k_steps=K_STEPS):
    import concourse.mybir as mybir
    import concourse.tile as tile
    from concourse import bacc
    from concourse.masks import make_identity
    from contextlib import ExitStack

    f32 = mybir.dt.float32
    f16 = mybir.dt.float16
    f8 = mybir.dt.float8e4
    AF = mybir.ActivationFunctionType
    ALU = mybir.AluOpType
    AX = mybir.AxisListType
    DR = mybir.MatmulPerfMode.DoubleRow

    nb = b_loc // P               # b-tiles per core
    nd = D // P                   # contraction chunks over D
    ns = s // 512                 # s-chunks of 512
    nt = s // P                   # s-tiles of 128
    ng = (4 * H) // 512           # gate chunks

    nc = bacc.Bacc("TRN2", target_bir_lowering=False, debug=False)

    f_x = nc.dram_tensor("f_x", [b_loc, D], f32, kind="ExternalInput")
    g_S = nc.dram_tensor("g_S", [s, D], f32, kind="ExternalInput")
    W_ih = nc.dram_tensor("W_ih", [4 * H, D], f32, kind="ExternalInput")
    W_hh = nc.dram_tensor("W_hh", [4 * H, 2 * H], f32, kind="ExternalInput")
    b_ih = nc.dram_tensor("b_ih", [4 * H], f32, kind="ExternalInput")
    b_hh = nc.dram_tensor("b_hh", [4 * H], f32, kind="ExternalInput")
    out = nc.dram_tensor("out", [b_loc, D], f32, kind="ExternalOutput")

    with tile.TileContext(nc) as tc, ExitStack() as ctx:
        const = ctx.enter_context(tc.tile_pool(name="const", bufs=1))
        g_T = const.tile([P, nd, s], f16)            # g_S.T resident
        whhT = const.tile([P, 2 * nd, 4 * H], f16)   # W_hh.T resident
        xw = const.tile([P, nb, 4 * H], f16)         # f_x@W_ih.T + biases
        fx32 = const.tile([P, nb, D], f32)
        br16 = const.tile([1, 4 * H], f16)
        ones16 = const.tile([1, P], f16)
        ident = const.tile([P, P], f16)
        ident8 = const.tile([P, P], f8)

        dram = ctx.enter_context(tc.tile_pool(name="dram", bufs=1, space="DRAM"))
        g8d = dram.tile([s, D], f8)                  # fp8 copy of g_S

        p_pool = ctx.enter_context(tc.tile_pool(name="p_pool", bufs=3))
        wst_pool = ctx.enter_context(tc.tile_pool(name="wst", bufs=2))
        gsb_pool = ctx.enter_context(tc.tile_pool(name="gsb", bufs=3))
        pt_pool = ctx.enter_context(tc.tile_pool(name="ptp", bufs=4))
        ht_pool = ctx.enter_context(tc.tile_pool(name="htp", bufs=7))
        rt_pool = ctx.enter_context(tc.tile_pool(name="rtp", bufs=2))
        rh_pool = ctx.enter_context(tc.tile_pool(name="rhp", bufs=2))
        lstm_pool = ctx.enter_context(tc.tile_pool(name="lstm", bufs=2))
        z_pool = ctx.enter_context(tc.tile_pool(name="zp", bufs=4))
        st_pool = ctx.enter_context(tc.tile_pool(name="stp", bufs=2))

        ps_log = ctx.enter_context(tc.tile_pool(name="ps_log", bufs=4, space="PSUM"))
        ps_g = ctx.enter_context(tc.tile_pool(name="ps_g", bufs=2, space="PSUM"))
        ps_tp = ctx.enter_context(tc.tile_pool(name="ps_tp", bufs=2, space="PSUM"))

        make_identity(nc, ident[:])
        make_identity(nc, ident8[:])

        _tpn = [0]

        def tp_group(blocks, dst, copy_engine="v", dt=f16):
            """PE-transpose len(blocks) [128,128] blocks into one PSUM
            group tile, then one (possibly strided) copy into dst
            (shape [P, len(blocks), P])."""
            n = len(blocks)
            _tpn[0] += 1
            if dt == f8:
                # fp8 transpose outputs must land with element step 2
                # (16-bit PE datapath); write through a strided view.
                tp = ps_tp.tile([P, n, 2 * P], f8, tag="tp",
                                name=f"tp_{_tpn[0]}")
                tv = tp[:].rearrange("p n (x two) -> p n x two", two=2)
                for t, blk in enumerate(blocks):
                    nc.tensor.transpose(tv[:, t, :, 0], blk, ident8[:])
                src = tv[:, :, :, 0]
            else:
                tp = ps_tp.tile([P, n, P], dt, tag="tp", name=f"tp_{_tpn[0]}")
                for t, blk in enumerate(blocks):
                    nc.tensor.transpose(tp[:, t, :], blk, ident[:])
                src = tp[:]
            if copy_engine == "v":
                nc.vector.tensor_copy(dst, src)
            elif copy_engine == "g":
                nc.gpsimd.tensor_copy(dst, src)
            else:
                nc.scalar.copy(dst, src)

        # ---------------- prolog: biases, f_x, W_ih/xw ----------------
        nc.vector.memset(ones16[:], 1.0)

        bi16 = wst_pool.tile([1, 4 * H], f16, tag="wst", name="bi16")
        bh16 = wst_pool.tile([1, 4 * H], f16, tag="wst", name="bh16")
        nc.gpsimd.dma_start(bi16[:], b_ih[:].rearrange("(a n) -> a n", a=1))
        nc.gpsimd.dma_start(bh16[:], b_hh[:].rearrange("(a n) -> a n", a=1))
        nc.vector.scalar_tensor_tensor(br16[:], bi16[:], 0.0, bh16[:],
                                       op0=ALU.add, op1=ALU.add)

        # f_x: f32 copy + fp16 copy + transposed fp16 tiles
        fx16 = wst_pool.tile([P, nb, D], f16, tag="wst", name="fx16")
        for j in range(nb):
            nc.sync.dma_start(fx32[:, j, :], f_x[j * P:(j + 1) * P, :])
            nc.gpsimd.dma_start(fx16[:, j, :], f_x[j * P:(j + 1) * P, :])
        hT = {}
        for j in range(nb):
            t = ht_pool.tile([P, nd, P], f16, tag="hT", name=f"fxT_{j}")
            tp_group([fx16[:, j, kk * P:(kk + 1) * P] for kk in range(nd)], t[:])
            hT[j] = t

        def load_wih(half):
            wtmp = wst_pool.tile([P, 8, D], f16, tag="wst", name=f"wtmp_{half}")
            nc.gpsimd.dma_start(
                wtmp[:], W_ih[half * 8 * P:(half + 1) * 8 * P, :].rearrange(
                    "(a p) d -> p a d", p=P))
            return wtmp

        def emit_xw_half(half, wtmp):
            """wihT transposes + xw chunks (2 gate-column chunks) for one
            W_ih half. Chunk n only needs wihT columns [512n, 512(n+1))."""
            wihT = p_pool.tile([P, nd, 8 * P], f16, tag="p", name=f"wihT_{half}")
            for i in range(8):
                tp_group([wtmp[:, i, kk * P:(kk + 1) * P] for kk in range(nd)],
                         wihT[:, :, i * P:(i + 1) * P],
                         copy_engine="v" if i % 2 == 0 else "s")
            for j in range(nb):
                for u in range(2):
                    n = half * 2 + u
                    gp = ps_g.tile([P, 512], f32, tag="psg", name=f"xwps_{j}_{n}")
                    nc.tensor.matmul(gp[:], ones16[:],
                                     br16[:, n * 512:(n + 1) * 512],
                                     start=True, stop=False)
                    for kk in range(nd):
                        nc.tensor.matmul(gp[:], hT[j][:, kk, :],
                                         wihT[:, kk, u * 512:(u + 1) * 512],
                                         start=False, stop=(kk == nd - 1))
                    nc.scalar.copy(xw[:, j, n * 512:(n + 1) * 512], gp[:])

        # ---------------- step state ----------------
        z = {}
        for j in range(nb):
            zt = z_pool.tile([P, D], f32, tag="z", name=f"z0_{j}")
            nc.vector.memset(zt[:], 0.0)
            z[j] = zt

        pbuf, negmaxes, sums, fcorr, rsum = {}, {}, {}, {}, {}

        def alloc_A(j):
            pbuf[j] = p_pool.tile([P, s], f8, tag="p", name=f"p_{j}")
            negmaxes[j] = st_pool.tile([P, ns], f32, tag="nmx", name=f"nmx_{j}")
            sums[j] = st_pool.tile([P, ns], f32, tag="sums", name=f"sums_{j}")

        def emit_A_chunk(j, c):
            """logits + negmax + exp for one 512-column chunk of b-tile j"""
            ps = ps_log.tile([P, 512], f32, tag="psl", name=f"psl_{j}_{c}")
            for kk in range(nd):
                nc.tensor.matmul(
                    ps[:], hT[j][:, kk, :],
                    g_T[:, kk, c * 512:(c + 1) * 512],
                    start=(kk == 0), stop=(kk == nd - 1))
            nc.vector.tensor_reduce(
                negmaxes[j][:, c:c + 1], ps[:],
                axis=AX.X, op=ALU.max, negate=True)
            nc.scalar.activation(
                pbuf[j][:, c * 512:(c + 1) * 512], ps[:],
                AF.Exp, bias=negmaxes[j][:, c:c + 1],
                accum_out=sums[j][:, c:c + 1])

        def emit_A(j):
            alloc_A(j)
            for c in range(ns):
                emit_A_chunk(j, c)

        def emit_fin(j):
            """global max, correction factors, 1/sum for b-tile j"""
            nm = st_pool.tile([P, 1], f32, tag="nm", name=f"nm_{j}")
            nc.vector.tensor_reduce(nm[:], negmaxes[j][:], axis=AX.X, op=ALU.min)
            delta = st_pool.tile([P, ns], f32, tag="delta", name=f"delta_{j}")
            # delta_i = m_i - m = -negmax_i + nm
            nc.vector.tensor_scalar(delta[:], negmaxes[j][:], -1.0, nm[:],
                                    op0=ALU.mult, op1=ALU.add)
            fc = st_pool.tile([P, ns], f32, tag="fc", name=f"fc_{j}")
            nc.scalar.activation(fc[:], delta[:], AF.Exp)
            fcorr[j] = fc
            ws = st_pool.tile([P, ns], f32, tag="ws", name=f"ws_{j}")
            nc.vector.scalar_tensor_tensor(ws[:], sums[j][:], 0.0, fc[:],
                                           op0=ALU.add, op1=ALU.mult)
            ssum = st_pool.tile([P, 1], f32, tag="ssum", name=f"ssum_{j}")
            nc.vector.tensor_reduce(ssum[:], ws[:], axis=AX.X, op=ALU.add)
            rs = st_pool.tile([P, 1], f32, tag="rs", name=f"rs_{j}")
            nc.vector.reciprocal(rs[:], ssum[:])
            rsum[j] = rs

        def emit_B(j, k):
            """rescale p, transpose, fp8 readout, gates, LSTM update"""
            # p *= exp(m_i - m), in place on fp8; split between DVE and ACT
            for i in range(ns):
                sl = pbuf[j][:, i * 512:(i + 1) * 512]
                if i % 3 == 2:
                    nc.scalar.mul(sl, sl, fcorr[j][:, i:i + 1])
                else:
                    nc.vector.tensor_scalar_mul(sl, sl, fcorr[j][:, i:i + 1])
            # readout r = p~ @ g (DoubleRow fp8, 2 s-tiles per matmul);
            # p transposed in groups of 4 via PE, 2 groups ahead of the mms
            rp = ps_g.tile([P, D], f32, tag="psg", name=f"psr_{j}")
            pTg = {}

            def tpg(ig):
                grp = pt_pool.tile([P, 4, P], f8, tag="pt", name=f"pt_{j}_{ig}")
                tp_group([pbuf[j][:, (ig * 4 + t) * P:(ig * 4 + t + 1) * P]
                          for t in range(4)], grp[:],
                         copy_engine="v" if ig % 2 == 0 else "s", dt=f8)
                pTg[ig] = grp

            gsbs = {}

            def gload(ig):
                gg = gsb_pool.tile([P, 4, D], f8, tag="gsb", name=f"gsb_{j}_{ig}")
                nc.sync.dma_start(
                    gg[:], g8d[ig * 4 * P:(ig + 1) * 4 * P, :].rearrange(
                        "(a p) d -> p a d", p=P))
                gsbs[ig] = gg

            tpg(0)
            tpg(1)
            gload(0)
            gload(1)
            for ig in range(nt // 4):
                if ig + 2 < nt // 4:
                    tpg(ig + 2)
                    gload(ig + 2)
                for u in range(2):
                    c = ig * 2 + u
                    nc.tensor.matmul(rp[:], pTg[ig][:, 2 * u:2 * u + 2, :],
                                     gsbs[ig][:, 2 * u:2 * u + 2, :],
                                     start=(c == 0), stop=(c == nt // 2 - 1),
                                     perf_mode=DR)
                del pTg[ig]
                del gsbs[ig]
            r16 = rh_pool.tile([P, D], f16, tag="r16", bufs=1, name=f"r16_{j}")
            nc.vector.tensor_scalar_mul(r16[:], rp[:], rsum[j][:])
            rT = rt_pool.tile([P, nd, P], f16, tag="rT", name=f"rT_{j}")
            tp_group([r16[:, kk * P:(kk + 1) * P] for kk in range(nd)], rT[:])
            # gates = xw + h@Whh_h.T + r@Whh_r.T, in PSUM-bank pairs with
            # the contraction loop outermost (shared lhsT)
            tt = [None] * ng
            for n in range(ng):
                gp = ps_g.tile([P, 512], f32, tag="psg", name=f"psg_{j}_{n}")
                for kk in range(nd):
                    nc.tensor.matmul(gp[:], hT[j][:, kk, :],
                                     whhT[:, kk, n * 512:(n + 1) * 512],
                                     start=(kk == 0), stop=False)
                for kk in range(nd):
                    nc.tensor.matmul(gp[:], rT[:, kk, :],
                                     whhT[:, nd + kk, n * 512:(n + 1) * 512],
                                     start=False, stop=(kk == nd - 1))
                pre = lstm_pool.tile([P, 512], f16, tag="pre", name=f"pre_{j}_{n}")
                nc.vector.scalar_tensor_tensor(
                    pre[:], gp[:], 0.0, xw[:, j, n * 512:(n + 1) * 512],
                    op0=ALU.add, op1=ALU.add)
                t = lstm_pool.tile([P, 512], f16, tag=f"t{n}", bufs=1,
                                   name=f"t{n}_{j}")
                # i,f,o gates: tanh(x/2) (-> sigmoid); g gate: tanh(x)
                nc.scalar.activation(t[:], pre[:], AF.Tanh,
                                     scale=1.0 if n == 2 else 0.5)
                tt[n] = t
            ti, tf, tg, to = tt
            # z' = 0.5*(tf+1)*z + (ti+1)*tg       (z = 2c)
            v = lstm_pool.tile([P, D], f16, tag="v", name=f"v_{j}")
            nc.vector.scalar_tensor_tensor(v[:], ti[:], 1.0, tg[:],
                                           op0=ALU.add, op1=ALU.mult)
            q = lstm_pool.tile([P, D], f16, tag="q", name=f"q_{j}")
            nc.vector.scalar_tensor_tensor(q[:], tf[:], 1.0, z[j][:],
                                           op0=ALU.add, op1=ALU.mult)
            zn = z_pool.tile([P, D], f32, tag="z", name=f"z_{j}")
            nc.vector.scalar_tensor_tensor(zn[:], q[:], 0.5, v[:],
                                           op0=ALU.mult, op1=ALU.add)
            z[j] = zn
            # h' = 0.5*(to+1)*tanh(z'/2) + f_x
            y = lstm_pool.tile([P, D], f16, tag="y", name=f"y_{j}")
            nc.scalar.activation(y[:], zn[:], AF.Tanh, scale=0.5)
            w = lstm_pool.tile([P, D], f16, tag="w", name=f"w_{j}")
            nc.vector.scalar_tensor_tensor(w[:], to[:], 1.0, y[:],
                                           op0=ALU.add, op1=ALU.mult)
            if k < k_steps - 1:
                h16 = rh_pool.tile([P, D], f16, tag="h16", bufs=1, name=f"h16_{j}")
                nc.vector.scalar_tensor_tensor(h16[:], w[:], 0.5, fx32[:, j, :],
                                               op0=ALU.mult, op1=ALU.add)
                hTn = ht_pool.tile([P, nd, P], f16, tag="hT", name=f"hT_{j}")
                tp_group([h16[:, kk * P:(kk + 1) * P] for kk in range(nd)], hTn[:])
                hT[j] = hTn
            else:
                ho = z_pool.tile([P, D], f32, tag="z", name=f"ho_{j}")
                nc.vector.scalar_tensor_tensor(ho[:], w[:], 0.5, fx32[:, j, :],
                                               op0=ALU.mult, op1=ALU.add)
                nc.sync.dma_start(out[j * P:(j + 1) * P, :], ho[:])

        # ---------------- interleaved prolog + step-0 A(0)/A(1) --------
        # g_S groups (4 s-tiles = one 512-chunk each): cast-load f16,
        # transpose into g_T, cast to fp8 + store to the DRAM scratch, and
        # emit the step-0 logits A-pair for b-tiles 0 and 1 every second
        # group. W_ih halves + xw are threaded between the early groups so
        # neither the PE nor the gpsimd DMA queue idles.
        def load_g(tg4):
            gt = wst_pool.tile([P, 8, D], f16, tag="wst", name=f"gload_{tg4}")
            nc.gpsimd.dma_start(
                gt[:, 0:4, :], g_S[tg4 * 4 * P:(tg4 + 1) * 4 * P, :].rearrange(
                    "(a p) d -> p a d", p=P))
            return gt

        def emit_g(tg4, gt):
            g8t = gsb_pool.tile([P, 4, D], f8, tag="gsb", name=f"g8t_{tg4}")
            if tg4 % 2 == 0:
                nc.vector.tensor_copy(g8t[:], gt[:, 0:4, :])
            else:
                nc.scalar.copy(g8t[:], gt[:, 0:4, :])
            nc.sync.dma_start(
                g8d[tg4 * 4 * P:(tg4 + 1) * 4 * P, :].rearrange(
                    "(a p) d -> p a d", p=P), g8t[:])
            for a in range(4):
                t = tg4 * 4 + a
                tp_group([gt[:, a, kk * P:(kk + 1) * P] for kk in range(nd)],
                         g_T[:, :, t * P:(t + 1) * P],
                         copy_engine="v" if t % 2 == 0 else "s")

        def emit_A_chunks_pro(c):
            emit_A_chunk(0, c)
            if nb > 1:
                emit_A_chunk(1, c)

        # gpsimd DMA queue: Wih0, g0, g1, Wih1, g2, g3, g4, ... ; the PE
        # consumes in the same order with xw/A-pairs threaded between.
        wtmp0 = load_wih(0)
        gt0 = load_g(0)
        gt1 = load_g(1)
        emit_xw_half(0, wtmp0)
        # pbuf/wihT share p_pool slots: pbufs must be allocated after
        # wihT_0 so the slot rotation frees wihT_0's slot via xw, not via
        # a step-0 B-phase.
        alloc_A(0)
        if nb > 1:
            alloc_A(1)
        wtmp1 = load_wih(1)
        emit_g(0, gt0)
        emit_A_chunks_pro(0)
        emit_g(1, gt1)
        emit_A_chunks_pro(1)
        gt2 = load_g(2)
        gt3 = load_g(3)
        emit_xw_half(1, wtmp1)
        emit_g(2, gt2)
        emit_A_chunks_pro(2)
        gts = {3: gt3}
        for tg4 in range(3, nt // 4):
            if tg4 + 1 < nt // 4:
                gts[tg4 + 1] = load_g(tg4 + 1)
            emit_g(tg4, gts.pop(tg4))
            emit_A_chunks_pro(tg4)

        # W_hh -> whhT fp16 (transposed); DMA had the whole g-loop to land
        for half in range(4):
            wh = wst_pool.tile([P, 8, D], f16, tag="wst", name=f"wh_{half}")
            whv = wh[:].rearrange("p (a b) d -> p a (b d)", b=2)
            nc.gpsimd.dma_start(
                whv,
                W_hh[half * 4 * P:(half + 1) * 4 * P, :].rearrange(
                    "(a p) d -> p a d", p=P))
            for i in range(4):
                tp_group([whv[:, i, kk * P:(kk + 1) * P] for kk in range(2 * nd)],
                         whhT[:, :, half * 4 * P + i * P:half * 4 * P + (i + 1) * P])

        # ---------------- K steps ----------------
        for k in range(k_steps):
            # schedule: A(0) A(1) B(0) A(2) B(1) A(3) B(2) B(3)
            # (step 0's A(0)/A(1) were emitted inside the prolog above)
            if k > 0:
                emit_A(0)
                emit_fin(0)
                if nb > 1:
                    emit_A(1)
                    emit_fin(1)
            else:
                emit_fin(0)
                if nb > 1:
                    emit_fin(1)
            emit_B(0, k)
            for j in range(2, nb):
                emit_A(j)
                emit_fin(j)
                emit_B(j - 1, k)
            if nb > 1:
                emit_B(nb - 1, k)

    return nc


_NC_CACHE = {}


def _get_nc():
    if "full" not in _NC_CACHE:
        nc = build_bass()
        nc.finalize()
        _NC_CACHE["full"] = nc
    return _NC_CACHE["full"]


def kernel(f_x, g_S, W_ih, W_hh, b_ih, b_hh):
    from concourse.bass_utils import run_bass_kernel_spmd

    nc = _get_nc()
    f_x = np.ascontiguousarray(f_x, dtype=np.float32)
    g_S = np.ascontiguousarray(g_S, dtype=np.float32)
    W_ih = np.ascontiguousarray(W_ih, dtype=np.float32)
    W_hh = np.ascontiguousarray(W_hh, dtype=np.float32)
    b_ih = np.ascontiguousarray(b_ih, dtype=np.float32)
    b_hh = np.ascontiguousarray(b_hh, dtype=np.float32)
    in_maps = [
        {
            "f_x": f_x[c * B_LOC:(c + 1) * B_LOC],
            "g_S": g_S,
            "W_ih": W_ih,
            "W_hh": W_hh,
            "b_ih": b_ih,
            "b_hh": b_hh,
        }
        for c in range(N_CORES)
    ]
    res = run_bass_kernel_spmd(nc, in_maps, core_ids=list(range(N_CORES)))
    return np.concatenate([res.results[c]["out"] for c in range(N_CORES)], axis=0)


if __name__ == "__main__":
    nc = build_bass()
    nc.finalize()
    print("built ok")


# revision 18
# speedup vs baseline: 1.0194x; 1.0194x over previous
"""Trainium2 Bass kernel for nn_AttLSTM (attention-LSTM, K=4 steps).

Math per step (reference):
    a = softmax(h @ g_S.T, axis=1)            # [B, S]
    r = a @ g_S                               # [B, D]
    gates = f_x @ W_ih.T + b_ih + [h, r] @ W_hh.T + b_hh
    i, f, g, o = split(gates, 4)
    c' = sig(f)*c + sig(i)*tanh(g); h' = sig(o)*tanh(c') + f_x

Design (per core, data-parallel over batch: B_loc = 512 rows/core):
  - logits/gates matmuls in fp16 (f32 PSUM accumulation); the attention
    readout r = p @ g in fp8e4 with MatmulPerfMode.DoubleRow (2 k-tiles
    per instruction, 2x PE rate). Softmax probabilities quantize to fp8
    benignly (peaked distribution: logit std ~ sqrt(D) = 22.6).
  - x @ W_ih.T + biases precomputed once (x == f_x every step) -> xw.
  - g_S kept two ways: transposed [D, S] resident in SBUF fp16 (g_T, rhs
    of the logits matmul) and natural [S, D] streamed per step from a
    DRAM fp8 scratch copy (rhs of the DoubleRow readout matmul).
  - ALL transposes via PE transpose-mode in groups of [128,128] blocks
    into one PSUM bank + one strided copy back to SBUF.
  - softmax per 128-row b-tile: per-512-chunk negated max (DVE, from
    PSUM), exp with per-chunk bias straight from PSUM (ACT, fp8 out) +
    accum_out row-sums, then a global per-row rescale p *= exp(m_chunk -
    m_row) before use.
  - logits/gates matmuls emitted in PSUM-bank *pairs* with the
    contraction loop outermost so consecutive instructions share lhsT
    (halves LDWEIGHTS traffic when the compiler reuses loaded weights).
  - prolog interleaved with step 0: the A-phase (logits+softmax) of
    b-tiles 0 and 1 is emitted chunk-by-chunk between the g_S group
    loads/transposes, so the PE never idles waiting for DMA.
  - sigmoid computed as 0.5*tanh(x/2)+0.5 so the single `exp_and_others`
    ACT table set (Exp + Tanh) serves the whole kernel.
  - LSTM pointwise math as fused scalar_tensor_tensor ops on DVE,
    carrying z = 2c as state.
"""

import os
import sys

import numpy as np

for _p in ("/opt/trn_rl_repo",):
    if _p not in sys.path and os.path.isdir(_p):
        sys.path.insert(0, _p)

# Problem sizes (hardcoded per spec).
B, S, D = 4096, 8192, 512
H = D
N_CORES = 8
B_LOC = B // N_CORES          # 512 rows per core
K_STEPS = 4
P = 128                       # partitions


def build_bass(b_loc=B_LOC, s=S, k_steps=K_STEPS):
    import concourse.mybir as mybir
    import concourse.tile as tile
    from concourse import bacc
    from concourse.masks import make_identity
    from contextlib import ExitStack

    f32 = mybir.dt.float32
    f16 = mybir.dt.float16
    f8 = mybir.dt.float8e4
    AF = mybir.ActivationFunctionType
    ALU = mybir.AluOpType
    AX = mybir.AxisListType
    DR = mybir.MatmulPerfMode.DoubleRow

    nb = b_loc // P               # b-tiles per core
    nd = D // P                   # contraction chunks over D
    ns = s // 512                 # s-chunks of 512
    nt = s // P                   # s-tiles of 128
    ng = (4 * H) // 512           # gate chunks

    nc = bacc.Bacc("TRN2", target_bir_lowering=False, debug=False)

    f_x = nc.dram_tensor("f_x", [b_loc, D], f32, kind="ExternalInput")
    g_S = nc.dram_tensor("g_S", [s, D], f32, kind="ExternalInput")
    W_ih = nc.dram_tensor("W_ih", [4 * H, D], f32, kind="ExternalInput")
    W_hh = nc.dram_tensor("W_hh", [4 * H, 2 * H], f32, kind="ExternalInput")
    b_ih = nc.dram_tensor("b_ih", [4 * H], f32, kind="ExternalInput")
    b_hh = nc.dram_tensor("b_hh", [4 * H], f32, kind="ExternalInput")
    out = nc.dram_tensor("out", [b_loc, D], f32, kind="ExternalOutput")

    with tile.TileContext(nc) as tc, ExitStack() as ctx:
        const = ctx.enter_context(tc.tile_pool(name="const", bufs=1))
        g_T = const.tile([P, nd, s], f16)            # g_S.T resident
        whhT = const.tile([P, 2 * nd, 4 * H], f16)   # W_hh.T resident
        xw = const.tile([P, nb, 4 * H], f16)         # f_x@W_ih.T + biases
        fx32 = const.tile([P, nb, D], f32)
        br16 = const.tile([1, 4 * H], f16)
        ones16 = const.tile([1, P], f16)
        ident = const.tile([P, P], f16)
        ident8 = const.tile([P, P], f8)

        dram = ctx.enter_context(tc.tile_pool(name="dram", bufs=1, space="DRAM"))
        g8d = dram.tile([s, D], f8)                  # fp8 copy of g_S

        p_pool = ctx.enter_context(tc.tile_pool(name="p_pool", bufs=3))
        wst_pool = ctx.enter_context(tc.tile_pool(name="wst", bufs=2))
        gsb_pool = ctx.enter_context(tc.tile_pool(name="gsb", bufs=3))
        pt_pool = ctx.enter_context(tc.tile_pool(name="ptp", bufs=4))
        ht_pool = ctx.enter_context(tc.tile_pool(name="htp", bufs=7))
        rt_pool = ctx.enter_context(tc.tile_pool(name="rtp", bufs=2))
        rh_pool = ctx.enter_context(tc.tile_pool(name="rhp", bufs=2))
        lstm_pool = ctx.enter_context(tc.tile_pool(name="lstm", bufs=2))
        z_pool = ctx.enter_context(tc.tile_pool(name="zp", bufs=4))
        st_pool = ctx.enter_context(tc.tile_pool(name="stp", bufs=2))

        ps_log = ctx.enter_context(tc.tile_pool(name="ps_log", bufs=4, space="PSUM"))
        ps_g = ctx.enter_context(tc.tile_pool(name="ps_g", bufs=2, space="PSUM"))
        ps_tp = ctx.enter_context(tc.tile_pool(name="ps_tp", bufs=2, space="PSUM"))

        make_identity(nc, ident[:])
        make_identity(nc, ident8[:])

        _tpn = [0]

        def tp_group(blocks, dst, copy_engine="v", dt=f16):
            """PE-transpose len(blocks) [128,128] blocks into one PSUM
            group tile, then one (possibly strided) copy into dst
            (shape [P, len(blocks), P])."""
            n = len(blocks)
            _tpn[0] += 1
            if dt == f8:
                # fp8 transpose outputs must land with element step 2
                # (16-bit PE datapath); write through a strided view.
                tp = ps_tp.tile([P, n, 2 * P], f8, tag="tp",
                                name=f"tp_{_tpn[0]}")
                tv = tp[:].rearrange("p n (x two) -> p n x two", two=2)
                for t, blk in enumerate(blocks):
                    nc.tensor.transpose(tv[:, t, :, 0], blk, ident8[:])
                src = tv[:, :, :, 0]
            else:
                tp = ps_tp.tile([P, n, P], dt, tag="tp", name=f"tp_{_tpn[0]}")
                for t, blk in enumerate(blocks):
                    nc.tensor.transpose(tp[:, t, :], blk, ident[:])
                src = tp[:]
            if copy_engine == "v":
                nc.vector.tensor_copy(dst, src)
            elif copy_engine == "g":
                nc.gpsimd.tensor_copy(dst, src)
            else:
                nc.scalar.copy(dst, src)

        # ---------------- prolog: biases, f_x, W_ih/xw ----------------
        nc.vector.memset(ones16[:], 1.0)

        bi16 = wst_pool.tile([1, 4 * H], f16, tag="wst", name="bi16")
        bh16 = wst_pool.tile([1, 4 * H], f16, tag="wst", name="bh16")
        nc.gpsimd.dma_start(bi16[:], b_ih[:].rearrange("(a n) -> a n", a=1))
        nc.gpsimd.dma_start(bh16[:], b_hh[:].rearrange("(a n) -> a n", a=1))
        nc.vector.scalar_tensor_tensor(br16[:], bi16[:], 0.0, bh16[:],
                                       op0=ALU.add, op1=ALU.add)

        # f_x: f32 copy + fp16 copy + transposed fp16 tiles
        fx16 = wst_pool.tile([P, nb, D], f16, tag="wst", name="fx16")
        for j in range(nb):
            nc.sync.dma_start(fx32[:, j, :], f_x[j * P:(j + 1) * P, :])
            nc.gpsimd.dma_start(fx16[:, j, :], f_x[j * P:(j + 1) * P, :])
        hT = {}
        for j in range(nb):
            t = ht_pool.tile([P, nd, P], f16, tag="hT", name=f"fxT_{j}")
            tp_group([fx16[:, j, kk * P:(kk + 1) * P] for kk in range(nd)], t[:])
            hT[j] = t

        def load_wih(half):
            wtmp = wst_pool.tile([P, 8, D], f16, tag="wst", name=f"wtmp_{half}")
            nc.gpsimd.dma_start(
                wtmp[:], W_ih[half * 8 * P:(half + 1) * 8 * P, :].rearrange(
                    "(a p) d -> p a d", p=P))
            return wtmp

        def emit_xw_half(half, wtmp):
            """wihT transposes + xw chunks (2 gate-column chunks) for one
            W_ih half. Chunk n only needs wihT columns [512n, 512(n+1))."""
            wihT = p_pool.tile([P, nd, 8 * P], f16, tag="p", name=f"wihT_{half}")
            for i in range(8):
                tp_group([wtmp[:, i, kk * P:(kk + 1) * P] for kk in range(nd)],
                         wihT[:, :, i * P:(i + 1) * P],
                         copy_engine="v" if i % 2 == 0 else "s")
            for j in range(nb):
                for u in range(2):
                    n = half * 2 + u
                    gp = ps_g.tile([P, 512], f32, tag="psg", name=f"xwps_{j}_{n}")
                    nc.tensor.matmul(gp[:], ones16[:],
                                     br16[:, n * 512:(n + 1) * 512],
                                     start=True, stop=False)
                    for kk in range(nd):
                        nc.tensor.matmul(gp[:], hT[j][:, kk, :],
                                         wihT[:, kk, u * 512:(u + 1) * 512],
                                         start=False, stop=(kk == nd - 1))
                    nc.scalar.copy(xw[:, j, n * 512:(n + 1) * 512], gp[:])

        # ---------------- step state ----------------
        z = {}
        for j in range(nb):
            zt = z_pool.tile([P, D], f32, tag="z", name=f"z0_{j}")
            nc.vector.memset(zt[:], 0.0)
            z[j] = zt

        pbuf, negmaxes, sums, fcorr, rsum = {}, {}, {}, {}, {}

        def alloc_A(j):
            pbuf[j] = p_pool.tile([P, s], f8, tag="p", name=f"p_{j}")
            negmaxes[j] = st_pool.tile([P, ns], f32, tag="nmx", name=f"nmx_{j}")
            sums[j] = st_pool.tile([P, ns], f32, tag="sums", name=f"sums_{j}")

        def emit_A_pair(j, cp):
            """logits + per-chunk negmax + exp for chunks (2cp, 2cp+1) of
            b-tile j. Contraction loop outermost over a PSUM-bank pair so
            consecutive matmuls alternate banks (no same-bank turnaround)."""
            ps = [ps_log.tile([P, 512], f32, tag="psl", name=f"psl_{j}_{cp}_{u}")
                  for u in range(2)]
            for kk in range(nd):
                for u in range(2):
                    c = 2 * cp + u
                    nc.tensor.matmul(
                        ps[u][:], hT[j][:, kk, :],
                        g_T[:, kk, c * 512:(c + 1) * 512],
                        start=(kk == 0), stop=(kk == nd - 1))
            for u in range(2):
                c = 2 * cp + u
                nc.vector.tensor_reduce(
                    negmaxes[j][:, c:c + 1], ps[u][:],
                    axis=AX.X, op=ALU.max, negate=True)
                nc.scalar.activation(
                    pbuf[j][:, c * 512:(c + 1) * 512], ps[u][:],
                    AF.Exp, bias=negmaxes[j][:, c:c + 1],
                    accum_out=sums[j][:, c:c + 1])

        def emit_A(j):
            alloc_A(j)
            for cp in range(ns // 2):
                emit_A_pair(j, cp)

        def emit_fin(j):
            """global max, correction factors, 1/sum for b-tile j"""
            nm = st_pool.tile([P, 1], f32, tag="nm", name=f"nm_{j}")
            nc.vector.tensor_reduce(nm[:], negmaxes[j][:], axis=AX.X, op=ALU.min)
            delta = st_pool.tile([P, ns], f32, tag="delta", name=f"delta_{j}")
            # delta_i = m_i - m = -negmax_i + nm
            nc.vector.tensor_scalar(delta[:], negmaxes[j][:], -1.0, nm[:],
                                    op0=ALU.mult, op1=ALU.add)
            fc = st_pool.tile([P, ns], f32, tag="fc", name=f"fc_{j}")
            nc.scalar.activation(fc[:], delta[:], AF.Exp)
            fcorr[j] = fc
            ws = st_pool.tile([P, ns], f32, tag="ws", name=f"ws_{j}")
            nc.vector.scalar_tensor_tensor(ws[:], sums[j][:], 0.0, fc[:],
                                           op0=ALU.add, op1=ALU.mult)
            ssum = st_pool.tile([P, 1], f32, tag="ssum", name=f"ssum_{j}")
            nc.vector.tensor_reduce(ssum[:], ws[:], axis=AX.X, op=ALU.add)
            rs = st_pool.tile([P, 1], f32, tag="rs", name=f"rs_{j}")
            nc.vector.reciprocal(rs[:], ssum[:])
            rsum[j] = rs

        def emit_B(j, k):
            """rescale p, transpose, fp8 readout, gates, LSTM update"""
            # p *= exp(m_i - m), in place on fp8; split between DVE and ACT
            for i in range(ns):
                sl = pbuf[j][:, i * 512:(i + 1) * 512]
                if i % 3 == 2:
                    nc.scalar.mul(sl, sl, fcorr[j][:, i:i + 1])
                else:
                    nc.vector.tensor_scalar_mul(sl, sl, fcorr[j][:, i:i + 1])
            # readout r = p~ @ g (DoubleRow fp8, 2 s-tiles per matmul);
            # p transposed in groups of 4 via PE, 2 groups ahead of the mms
            rp = ps_g.tile([P, D], f32, tag="psg", name=f"psr_{j}")
            pTg = {}

            def tpg(ig):
                grp = pt_pool.tile([P, 4, P], f8, tag="pt", name=f"pt_{j}_{ig}")
                tp_group([pbuf[j][:, (ig * 4 + t) * P:(ig * 4 + t + 1) * P]
                          for t in range(4)], grp[:],
                         copy_engine="v" if ig % 2 == 0 else "s", dt=f8)
                pTg[ig] = grp

            gsbs = {}

            def gload(ig):
                gg = gsb_pool.tile([P, 4, D], f8, tag="gsb", name=f"gsb_{j}_{ig}")
                nc.sync.dma_start(
                    gg[:], g8d[ig * 4 * P:(ig + 1) * 4 * P, :].rearrange(
                        "(a p) d -> p a d", p=P))
                gsbs[ig] = gg

            tpg(0)
            tpg(1)
            gload(0)
            gload(1)
            for ig in range(nt // 4):
                if ig + 2 < nt // 4:
                    tpg(ig + 2)
                    gload(ig + 2)
                for u in range(2):
                    c = ig * 2 + u
                    nc.tensor.matmul(rp[:], pTg[ig][:, 2 * u:2 * u + 2, :],
                                     gsbs[ig][:, 2 * u:2 * u + 2, :],
                                     start=(c == 0), stop=(c == nt // 2 - 1),
                                     perf_mode=DR)
                del pTg[ig]
                del gsbs[ig]
            r16 = rh_pool.tile([P, D], f16, tag="r16", bufs=1, name=f"r16_{j}")
            nc.vector.tensor_scalar_mul(r16[:], rp[:], rsum[j][:])
            rT = rt_pool.tile([P, nd, P], f16, tag="rT", name=f"rT_{j}")
            tp_group([r16[:, kk * P:(kk + 1) * P] for kk in range(nd)], rT[:])
            # gates = xw + h@Whh_h.T + r@Whh_r.T, in PSUM-bank pairs with
            # the contraction loop outermost (shared lhsT)
            tt = [None] * ng
            for half in range(2):
                gp = [ps_g.tile([P, 512], f32, tag="psg",
                                name=f"psg_{j}_{half}_{u}") for u in range(2)]
                for kk in range(nd):
                    for u in range(2):
                        n = half * 2 + u
                        nc.tensor.matmul(gp[u][:], hT[j][:, kk, :],
                                         whhT[:, kk, n * 512:(n + 1) * 512],
                                         start=(kk == 0), stop=False)
                for kk in range(nd):
                    for u in range(2):
                        n = half * 2 + u
                        nc.tensor.matmul(gp[u][:], rT[:, kk, :],
                                         whhT[:, nd + kk, n * 512:(n + 1) * 512],
                                         start=False, stop=(kk == nd - 1))
                for u in range(2):
                    n = half * 2 + u
                    pre = lstm_pool.tile([P, 512], f16, tag="pre",
                                         name=f"pre_{j}_{n}")
                    nc.vector.scalar_tensor_tensor(
                        pre[:], gp[u][:], 0.0, xw[:, j, n * 512:(n + 1) * 512],
                        op0=ALU.add, op1=ALU.add)
                    t = lstm_pool.tile([P, 512], f16, tag=f"t{n}", bufs=1,
                                       name=f"t{n}_{j}")
                    # i,f,o gates: tanh(x/2) (-> sigmoid); g gate: tanh(x)
                    nc.scalar.activation(t[:], pre[:], AF.Tanh,
                                         scale=1.0 if n == 2 else 0.5)
                    tt[n] = t
            ti, tf, tg, to = tt
            # z' = 0.5*(tf+1)*z + (ti+1)*tg       (z = 2c)
            v = lstm_pool.tile([P, D], f16, tag="v", name=f"v_{j}")
            nc.vector.scalar_tensor_tensor(v[:], ti[:], 1.0, tg[:],
                                           op0=ALU.add, op1=ALU.mult)
            q = lstm_pool.tile([P, D], f16, tag="q", name=f"q_{j}")
            nc.vector.scalar_tensor_tensor(q[:], tf[:], 1.0, z[j][:],
                                           op0=ALU.add, op1=ALU.mult)
            zn = z_pool.tile([P, D], f32, tag="z", name=f"z_{j}")
            nc.vector.scalar_tensor_tensor(zn[:], q[:], 0.5, v[:],
                                           op0=ALU.mult, op1=ALU.add)
            z[j] = zn
            # h' = 0.5*(to+1)*tanh(z'/2) + f_x
            y = lstm_pool.tile([P, D], f16, tag="y", name=f"y_{j}")
            nc.scalar.activation(y[:], zn[:], AF.Tanh, scale=0.5)
            w = lstm_pool.tile([P, D], f16, tag="w", name=f"w_{j}")
            nc.vector.scalar_tensor_tensor(w[:], to[:], 1.0, y[:],
                                           op0=ALU.add, op1=ALU.mult)
            if k < k_steps - 1:
                h16 = rh_pool.tile([P, D], f16, tag="h16", bufs=1, name=f"h16_{j}")
                nc.vector.scalar_tensor_tensor(h16[:], w[:], 0.5, fx32[:, j, :],
                                               op0=ALU.mult, op1=ALU.add)
                hTn = ht_pool.tile([P, nd, P], f16, tag="hT", name=f"hT_{j}")
                tp_group([h16[:, kk * P:(kk + 1) * P] for kk in range(nd)], hTn[:])
                hT[j] = hTn
            else:
                ho = z_pool.tile([P, D], f32, tag="z", name=f"ho_{j}")
                nc.vector.scalar_tensor_tensor(ho[:], w[:], 0.5, fx32[:, j, :],
                                               op0=ALU.mult, op1=ALU.add)
                nc.sync.dma_start(out[j * P:(j + 1) * P, :], ho[:])

        # ---------------- interleaved prolog + step-0 A(0)/A(1) --------
        # g_S groups (4 s-tiles = one 512-chunk each): cast-load f16,
        # transpose into g_T, cast to fp8 + store to the DRAM scratch, and
        # emit the step-0 logits A-pair for b-tiles 0 and 1 every second
        # group. W_ih halves + xw are threaded between the early groups so
        # neither the PE nor the gpsimd DMA queue idles.
        def load_g(tg4):
            gt = wst_pool.tile([P, 8, D], f16, tag="wst", name=f"gload_{tg4}")
            nc.gpsimd.dma_start(
                gt[:, 0:4, :], g_S[tg4 * 4 * P:(tg4 + 1) * 4 * P, :].rearrange(
                    "(a p) d -> p a d", p=P))
            return gt

        def emit_g(tg4, gt):
            g8t = gsb_pool.tile([P, 4, D], f8, tag="gsb", name=f"g8t_{tg4}")
            if tg4 % 2 == 0:
                nc.vector.tensor_copy(g8t[:], gt[:, 0:4, :])
            else:
                nc.scalar.copy(g8t[:], gt[:, 0:4, :])
            nc.sync.dma_start(
                g8d[tg4 * 4 * P:(tg4 + 1) * 4 * P, :].rearrange(
                    "(a p) d -> p a d", p=P), g8t[:])
            for a in range(4):
                t = tg4 * 4 + a
                tp_group([gt[:, a, kk * P:(kk + 1) * P] for kk in range(nd)],
                         g_T[:, :, t * P:(t + 1) * P],
                         copy_engine="v" if t % 2 == 0 else "s")

        def emit_A_pairs_pro(cp):
            emit_A_pair(0, cp)
            if nb > 1:
                emit_A_pair(1, cp)

        # gpsimd DMA queue: Wih0, g0, g1, Wih1, g2, g3, g4, ... ; the PE
        # consumes in the same order with xw/A-pairs threaded between.
        wtmp0 = load_wih(0)
        gt0 = load_g(0)
        gt1 = load_g(1)
        emit_xw_half(0, wtmp0)
        # pbuf/wihT share p_pool slots: pbufs must be allocated after
        # wihT_0 so the slot rotation frees wihT_0's slot via xw, not via
        # a step-0 B-phase.
        alloc_A(0)
        if nb > 1:
            alloc_A(1)
        wtmp1 = load_wih(1)
        emit_g(0, gt0)
        emit_g(1, gt1)
        emit_A_pairs_pro(0)
        gt2 = load_g(2)
        gt3 = load_g(3)
        emit_xw_half(1, wtmp1)
        emit_g(2, gt2)
        gts = {3: gt3}
        for tg4 in range(3, nt // 4):
            if tg4 + 1 < nt // 4:
                gts[tg4 + 1] = load_g(tg4 + 1)
            emit_g(tg4, gts.pop(tg4))
            if tg4 % 2 == 1:
                emit_A_pairs_pro(tg4 // 2)

        # W_hh -> whhT fp16 (transposed); DMA had the whole g-loop to land
        for half in range(4):
            wh = wst_pool.tile([P, 8, D], f16, tag="wst", name=f"wh_{half}")
            whv = wh[:].rearrange("p (a b) d -> p a (b d)", b=2)
            nc.gpsimd.dma_start(
                whv,
                W_hh[half * 4 * P:(half + 1) * 4 * P, :].rearrange(
                    "(a p) d -> p a d", p=P))
            for i in range(4):
                tp_group([whv[:, i, kk * P:(kk + 1) * P] for kk in range(2 * nd)],
                         whhT[:, :, half * 4 * P + i * P:half * 4 * P + (i + 1) * P])

        # ---------------- K steps ----------------
        for k in range(k_steps):
            # schedule: A(0) A(1) B(0) A(2) B(1) A(3) B(2) B(3)
            # (step 0's A(0)/A(1) were emitted inside the prolog above)
            if k > 0:
                emit_A(0)
                emit_fin(0)
                if nb > 1:
                    emit_A(1)
                    emit_fin(1)
            else:
                emit_fin(0)
                if nb > 1:
                    emit_fin(1)
            emit_B(0, k)
            for j in range(2, nb):
                emit_A(j)
                emit_fin(j)
                emit_B(j - 1, k)
            if nb > 1:
                emit_B(nb - 1, k)

    return nc


_NC_CACHE = {}


def _get_nc():
    if "full" not in _NC_CACHE:
        nc = build_bass()
        nc.finalize()
        _NC_CACHE["full"] = nc
    return _NC_CACHE["full"]


def kernel(f_x, g_S, W_ih, W_hh, b_ih, b_hh):
    from concourse.bass_utils import run_bass_kernel_spmd

    nc = _get_nc()
    f_x = np.ascontiguousarray(f_x, dtype=np.float32)
    g_S = np.ascontiguousarray(g_S, dtype=np.float32)
    W_ih = np.ascontiguousarray(W_ih, dtype=np.float32)
    W_hh = np.ascontiguousarray(W_hh, dtype=np.float32)
    b_ih = np.ascontiguousarray(b_ih, dtype=np.float32)
    b_hh = np.ascontiguousarray(b_hh, dtype=np.float32)
    in_maps = [
        {
            "f_x": f_x[c * B_LOC:(c + 1) * B_LOC],
            "g_S": g_S,
            "W_ih": W_ih,
            "W_hh": W_hh,
            "b_ih": b_ih,
            "b_hh": b_hh,
        }
        for c in range(N_CORES)
    ]
    res = run_bass_kernel_spmd(nc, in_maps, core_ids=list(range(N_CORES)))
    return np.concatenate([res.results[c]["out"] for c in range(N_CORES)], axis=0)


if __name__ == "__main__":
    nc = build_bass()
    nc.finalize()
    print("built ok")
